# revision 7
# baseline (speedup 1.0000x reference)
"""ArcFace logits kernel for 8 TRN2 NeuronCores (class-parallel / Partial-FC style).

Full computation:
    en = l2norm_rows(embeddings)           # [B, E]
    wn = l2norm_cols(w)                    # [E, C]
    cos = clip(en @ wn, -1+1e-6, 1-1e-6)   # [B, C]
    logits = 64 * where(onehot(labels), margin(cos), cos)

Distribution: class dim C=100000 sharded 12500-per-core (padded to 12544 =
98*128). Embeddings replicated. Each core computes its logits shard
transposed ([C_shard, B]: per-column norm scale is a per-partition scalar).

v4 structure (v1 baseline measured 138 us):
- host ships the raw TRANSPOSED embeddings embT [E,B] bf16 (pure
  formatting); no on-device transposes. Row norms are computed FROM embT:
  squared (fp16) then column-summed by ones-stationary matmuls into a
  [1,B] PSUM row; 1/sqrt of that row is partition-broadcast by a tiny
  K=1 matmul into rnB [128,B], and the moving operand eTn = embT * rnB
  is built by 4 DVE multiplies. Group-0 matmuls start on the RAW embT
  (~3us in) with the row norm folded into their drains; all later groups
  stream eTn with plain per-partition-scaled drains.
- one merged DMA per logical transfer: the HWDGE descriptor engine costs
  ~0.6us per DMA serialized across queues, so DMA COUNT is a first-class
  resource (13 HWDGE DMAs vs 86 in v1). Output stores go through the
  otherwise-idle GPSIMD's SWDGE queue.
- w streams in ramped groups [2,4,8,14...] of C-tiles, 3-deep ring, one
  DMA per group.
- column norms: fp16 squared weights (2x DVE throughput vs v1's fp8, and
  better precision); the 4-tiny-MM-per-C-tile partition reductions are
  self-hosted in each group's first tiles, with drains lagging behind a
  6-deep PSUM ring until the group's scale vector is ready.
- margin path: elementwise prod_k = wlab_k * embT_k (fp16) column-summed
  by ones-stationary matmuls into [1,B] PSUM rows, rearranged by a tiny
  DMA into [128,4] for the margin math; cos scaled by s_wl and rn.

dtype: matmuls bf16 with f32 PSUM accumulation; logits bf16. Dense clip
skipped (|cos| << 1-1e-6 for this distribution; the margin path applies
clip exactly). End-to-end rel err vs the f32 reference ~3e-3 (tol 2e-2).
"""

import math
import os
from contextlib import ExitStack

ABLATE = set(os.environ.get("ABLATE4", "").split(","))

import ml_dtypes
import numpy as np

import concourse.bass as bass
import concourse.tile as tile
from concourse import bacc, mybir
from concourse.bass import ts
from concourse.bass_utils import run_bass_kernel_spmd

F32 = mybir.dt.float32
BF16 = mybir.dt.bfloat16
FP16 = mybir.dt.float16
AF = mybir.ActivationFunctionType
ALU = mybir.AluOpType

B = 512          # batch
E = 512          # embedding dim
C = 100000       # classes
NCORES = 8
CSH = C // NCORES          # 12500 real shard width
CSP = 12544                # padded shard width = 98 * 128
NT = CSP // 128            # 98 C-tiles of 128
NK = E // 128              # 4 contraction blocks

GROUPS = [2, 4, 8, 14, 14, 14, 14, 14, 14]   # C-tiles per w-load group
assert sum(GROUPS) == NT
NG = len(GROUPS)
GSTART = [0]
for t in GROUPS:
    GSTART.append(GSTART[-1] + t)
RAW_GROUPS = {0}          # groups whose matmuls stream raw embT (rn in drain)

MARGIN_G = 4     # emit margin block inside this group's tile loop
MARGIN_J = 7

MARGIN = 0.5
SCALE = 64.0
COS_M = math.cos(MARGIN)
SIN_M = math.sin(MARGIN)
TH = math.cos(math.pi - MARGIN)
MM = math.sin(MARGIN) * MARGIN
CLIP_EPS = 1e-6
NORM_EPS = 1e-12


def _self_host_plan(tg):
    """Host this group's tg tiny-MM groups on its own first ceil(tg/4)*... tiles.

    Returns [(host_local_j, jj), ...]: up to 4 tiny groups per host tile,
    spread over the first ceil(tg/4) tiles.
    """
    per = min(4, tg)
    return [(jj // per, jj) for jj in range(tg)]


def _out_splits(tg, last):
    """Split a group's tiles into out-store staging chunks (<=8 tiles each)."""
    splits = []
    rem = tg
    while rem > 8:
        splits.append(7)
        rem -= 7
    splits.append(rem)
    if last:
        tail = splits.pop()
        if tail > 3:
            splits.extend([tail - 3, 2, 1])
        elif tail > 1:
            splits.extend([tail - 1, 1])
        else:
            splits.append(1)
    return splits


def _make_pools(ctx, tc):
    p = {}
    p["sm"] = ctx.enter_context(tc.tile_pool(name="sm", bufs=1))
    p["pw"] = ctx.enter_context(tc.tile_pool(name="pw", bufs=3))
    p["pw2"] = ctx.enter_context(tc.tile_pool(name="pw2", bufs=2))
    p["psd"] = ctx.enter_context(tc.tile_pool(name="psd", bufs=2))
    p["pout"] = ctx.enter_context(tc.tile_pool(name="pout", bufs=3))
    p["psm"] = ctx.enter_context(tc.tile_pool(name="psm", bufs=6, space="PSUM"))
    p["psr"] = ctx.enter_context(tc.tile_pool(name="psr", bufs=1, space="PSUM"))
    p["pscn"] = ctx.enter_context(tc.tile_pool(name="pscn", bufs=1, space="PSUM"))
    return p


def _build_graph(p, tc, nc, embT, wsh, wlab, out, mv, rnscr, pfscr):
    p_sm = p["sm"]

    # --- constants ---
    ones16 = p_sm.tile([128, 1], FP16)
    nc.vector.memset(ones16[:], 1.0)
    onesB = p_sm.tile([1, 128], BF16)
    nc.vector.memset(onesB[:], 1.0)
    eps1 = p_sm.tile([1, 1], F32)
    nc.vector.memset(eps1[:], NORM_EPS)
    eps128 = p_sm.tile([128, 1], F32)
    nc.vector.memset(eps128[:], NORM_EPS)
    # first ACT instruction is a Sqrt so the table pass loads the combined
    # sqrt table (copy+square+sqrt) once instead of reloading mid-kernel
    warm = p_sm.tile([1, 1], F32)
    nc.scalar.activation(warm[:], eps1[:], AF.Sqrt)

    # --- merged DMA prologue ---
    # eTr: raw transposed embeddings [128, NK*B]; slice k = embT[k*128:(k+1)*128, :]
    eTr_all = p_sm.tile([128, NK * B], BF16)
    nc.scalar.dma_start(
        eTr_all[:].rearrange("q (k b) -> q k b", k=NK),
        embT[:, :].rearrange("(k q) b -> q k b", q=128),
    )
    eTr = [eTr_all[:, ts(k, B)] for k in range(NK)]

    wch = {}      # g -> w_all tile [128, NK*width]

    def issue_w_dma(g):
        c0, c1 = GSTART[g] * 128, GSTART[g + 1] * 128
        w_all = p["pw"].tile([128, NK * (c1 - c0)], BF16, name="w_all")
        nc.sync.dma_start(
            w_all[:].rearrange("q (k c) -> q k c", k=NK),
            wsh[:, c0:c1].rearrange("(k q) c -> q k c", q=128),
        )
        wch[g] = w_all

    for g in (0, 1, 2):
        issue_w_dma(g)

    def wsl(g, k, j):
        width = GROUPS[g] * 128
        return wch[g][:, k * width + j * 128 : k * width + (j + 1) * 128]

    # --- row norms from embT: rnB[p, b] = 1/||emb row b||, and eTn = embT*rnB ---
    # squares (fp16, split DVE/ACT), ones-stationary column sums -> [1, B]
    esq = p_sm.tile([128, NK * B], FP16)
    nc.vector.scalar_tensor_tensor(
        esq[:, : 2 * B], eTr_all[:, : 2 * B], 1.0, eTr_all[:, : 2 * B],
        op0=ALU.mult, op1=ALU.mult,
    )
    nc.scalar.activation(esq[:, 2 * B :], eTr_all[:, 2 * B :], AF.Square)
    psE = p["psr"].tile([128, B], F32, name="psr")
    for k in range(NK):
        nc.tensor.matmul(psE[0:1, :], ones16[:], esq[:, ts(k, B)],
                         start=(k == 0), stop=(k == NK - 1))
    rowE = p_sm.tile([1, B], F32)
    nc.scalar.activation(rowE[:], psE[0:1, :], AF.Sqrt, bias=eps1[:])
    rowR = p_sm.tile([1, B], F32)
    nc.vector.reciprocal(rowR[:], rowE[:])
    rowRb = p_sm.tile([1, B], BF16)
    nc.vector.tensor_copy(rowRb[:], rowR[:])
    rnB = p_sm.tile([128, B], F32)
    eTn_all = p_sm.tile([128, NK * B], BF16)
    eTn = [eTn_all[:, ts(k, B)] for k in range(NK)]

    def emit_norm_part2():
        psR = p["psr"].tile([128, B], F32, name="psr")
        nc.tensor.matmul(psR[:], onesB[:], rowRb[:], start=True, stop=True)
        nc.vector.tensor_copy(rnB[:], psR[:])
        for k in range(NK):
            nc.vector.tensor_mul(eTn_all[:, ts(k, B)], eTr[k], rnB[:])

    # margin-layout copy of the row norms: rn[q, m] = rowR[0, m*128+q]
    # (via DRAM: synthesizing a partition dim from SBUF row bytes miscompiles
    # on hardware, so store the row and reload it rearranged)
    nc.sync.dma_start(rnscr[:, :], rowR[:])
    rn = p_sm.tile([128, NK], F32)
    nc.sync.dma_start(
        rn[:], rnscr[:, :].rearrange("o (m q) -> (o q) m", q=128)
    )

    # --- column-norm machinery (fp16 squared weights) ---
    s_dense = p_sm.tile([128, NT], F32)
    w2ch = {}

    def emit_squares(g):
        if "notiny" in ABLATE:
            return
        width = GROUPS[g] * 128
        w2_all = p["pw2"].tile([128, NK * width], FP16, name="w2_all")
        nc.vector.scalar_tensor_tensor(
            w2_all[:, : 2 * width], wch[g][:, : 2 * width], 1.0,
            wch[g][:, : 2 * width], op0=ALU.mult, op1=ALU.mult,
        )
        nc.scalar.activation(
            w2_all[:, 2 * width :], wch[g][:, 2 * width :], AF.Square
        )
        w2ch[g] = w2_all

    pscn_t = {}

    def emit_tiny(g, jj):
        if "notiny" in ABLATE:
            return
        if jj == 0:
            pscn_t[g] = p["pscn"].tile([128, GROUPS[g]], F32, name="pscn")
        width = GROUPS[g] * 128
        for k in range(NK):
            nc.tensor.matmul(
                pscn_t[g][:, jj : jj + 1],
                w2ch[g][:, k * width + jj * 128 : k * width + (jj + 1) * 128],
                ones16[:],
                start=(k == 0), stop=(k == NK - 1),
            )

    def emit_scale(g):
        if "notiny" in ABLATE:
            if g == 0:
                nc.vector.memset(s_dense[:], 1.0)
            return
        ssq = p["psd"].tile([128, GROUPS[g]], F32, name="ssq")
        nc.scalar.activation(
            ssq[:], pscn_t[g][:], AF.Sqrt, scale=1.0 / (SCALE * SCALE),
            bias=eps128[:],
        )
        nc.vector.reciprocal(s_dense[:, GSTART[g] : GSTART[g + 1]], ssq[:])

    wl_all = p_sm.tile([128, NK * B], BF16)

    def load_wlab():
        nc.scalar.dma_start(
            wl_all[:].rearrange("q (k b) -> q k b", k=NK),
            wlab[:, :].rearrange("(k q) b -> q k b", q=128),
        )

    def emit_margin():
        # cos at label columns: psA[b] = sum_e wl[e,b]*embT[e,b] (raw),
        # psB[b] = sum_e wl[e,b]^2; cos = psA * rsqrt(psB) * rn
        prod_all = p_sm.tile([128, NK * B], FP16)
        wl2_all = p_sm.tile([128, NK * B], FP16)
        for k in range(NK):
            nc.vector.tensor_mul(
                prod_all[:, ts(k, B)], wl_all[:, ts(k, B)], eTr[k]
            )
            nc.scalar.activation(
                wl2_all[:, ts(k, B)], wl_all[:, ts(k, B)], AF.Square
            )
        psA = p["psr"].tile([128, B], F32, name="psr")
        for k in range(NK):
            nc.tensor.matmul(psA[0:1, :], ones16[:], prod_all[:, ts(k, B)],
                             start=(k == 0), stop=(k == NK - 1))
        rowAB = p_sm.tile([1, 2 * B], F32)
        nc.scalar.activation(rowAB[:, :B], psA[0:1, :], AF.Copy)
        psB = p["psr"].tile([128, B], F32, name="psr")
        for k in range(NK):
            nc.tensor.matmul(psB[0:1, :], ones16[:], wl2_all[:, ts(k, B)],
                             start=(k == 0), stop=(k == NK - 1))
        nc.vector.tensor_copy(rowAB[:, B:], psB[0:1, :])
        nc.sync.dma_start(pfscr[:, :], rowAB[:])
        pf = p_sm.tile([128, 2 * NK], F32)
        nc.sync.dma_start(
            pf[:], pfscr[:, :].rearrange("o (m q) -> (o q) m", q=128)
        )
        psA_r, psB_r = pf[:, :NK], pf[:, NK:]

        swl_s = p_sm.tile([128, NK], F32)
        nc.scalar.activation(swl_s[:], psB_r, AF.Sqrt, bias=eps128[:])
        s_wl = p_sm.tile([128, NK], F32)
        nc.vector.reciprocal(s_wl[:], swl_s[:])
        cosu = p_sm.tile([128, NK], F32)
        nc.vector.tensor_mul(cosu[:], psA_r, s_wl[:])
        cos_lab = p_sm.tile([128, NK], F32)
        nc.vector.tensor_mul(cos_lab[:], cosu[:], rn[:])

        cc = p_sm.tile([128, NK], F32)
        nc.vector.tensor_scalar_min(cc[:], cos_lab[:], 1.0 - CLIP_EPS)
        nc.vector.tensor_scalar_max(cc[:], cc[:], -1.0 + CLIP_EPS)
        c2 = p_sm.tile([128, NK], F32)
        nc.scalar.activation(c2[:], cc[:], AF.Square)
        sinv = p_sm.tile([128, NK], F32)
        nc.scalar.activation(sinv[:], c2[:], AF.Sqrt, scale=-1.0, bias=1.0)
        t1 = p_sm.tile([128, NK], F32)
        nc.vector.tensor_scalar_mul(t1[:], cc[:], COS_M)
        cm = p_sm.tile([128, NK], F32)
        nc.vector.scalar_tensor_tensor(
            cm[:], sinv[:], -SIN_M, t1[:], op0=ALU.mult, op1=ALU.add
        )
        alt = p_sm.tile([128, NK], F32)
        nc.vector.tensor_scalar_sub(alt[:], cc[:], MM)
        mk = p_sm.tile([128, NK], mybir.dt.int32)
        nc.vector.tensor_scalar(mk[:], cc[:], TH, None, op0=ALU.is_gt)
        res = p_sm.tile([128, NK], F32)
        nc.vector.tensor_copy(res[:], alt[:])
        nc.vector.copy_predicated(res[:], mk[:], cm[:])
        mvt = p_sm.tile([128, NK], F32)
        nc.vector.tensor_scalar_mul(mvt[:], res[:], SCALE)
        nc.sync.dma_start(mv[:, :], mvt[:])

    # --- main tile loop over groups ---
    for g in range(NG):
        tg = GROUPS[g]
        plan = _self_host_plan(tg)
        last_jj = plan[-1][1]
        if g == 1:
            load_wlab()
        if g >= 1 and g + 2 < NG:
            issue_w_dma(g + 2)
        moving = eTr if g in RAW_GROUPS else eTn
        splits = _out_splits(tg, g == NG - 1)
        split_edge = []
        acc = 0
        for s in splits:
            acc += s
            split_edge.append(acc)
        # emission: all matmuls (+tinies/scale/margin) first; drains and
        # stores are emitted at group end so every drain follows its
        # group's scale chain in engine order (execution still overlaps:
        # drains start as soon as scale + their psm are ready, and the
        # 6-deep psm ring lets matmuls run ahead).
        psms = []
        ots = []
        seg = 0
        for j in range(tg):
            if j == 0 or j == split_edge[seg] - splits[seg]:
                pass
            psm = p["psm"].tile([128, B], F32, name="psm")
            psms.append(psm)
            for k in range(NK):
                nc.tensor.matmul(
                    psm[:], wsl(g, k, j), moving[k],
                    start=(k == 0), stop=(k == NK - 1),
                )
            if j == 0:
                emit_squares(g)
            for hj, jj in plan:
                if hj == j:
                    emit_tiny(g, jj)
                    if jj == last_jj:
                        emit_scale(g)
            if g == MARGIN_G and j == MARGIN_J:
                emit_margin()
        if g == 0:
            emit_norm_part2()
        seg_base = 0
        for seg, seg_w in enumerate(splits):
            ot = p["pout"].tile([128, 8 * B], BF16, name="ot")
            for j in range(seg_base, seg_base + seg_w):
                t = GSTART[g] + j
                half = ot[:, ts(j - seg_base, B)]
                if g in RAW_GROUPS:
                    # raw moving operand: fold the row norm into the drain
                    nc.vector.scalar_tensor_tensor(
                        half, psms[j][:], s_dense[:, t : t + 1], rnB[:],
                        op0=ALU.mult, op1=ALU.mult,
                    )
                elif j % 2 == 0:
                    nc.scalar.activation(half, psms[j][:], AF.Copy,
                                         scale=s_dense[:, t : t + 1])
                else:
                    nc.vector.tensor_scalar_mul(half, psms[j][:],
                                                s_dense[:, t : t + 1])
            t0 = GSTART[g] + seg_base
            dst = out[t0 * 128 : (t0 + seg_w) * 128, :].rearrange(
                "(i q) b -> q i b", q=128
            )
            srcv = ot[:, : seg_w * B].rearrange("q (i b) -> q i b", i=seg_w)
            if "hwdgestores" in ABLATE:
                nc.scalar.dma_start(dst, srcv)
            elif g == NG - 1 and seg >= len(splits) - 2:
                nc.sync.dma_start(dst, srcv)
            else:
                nc.gpsimd.dma_start(dst, srcv)
            seg_base += seg_w


_NC_CACHE = {}


def _build(reps=1):
    """Build + compile. reps>1 wraps the whole body in a HW loop (for timing)."""
    if reps in _NC_CACHE:
        return _NC_CACHE[reps]
    nc = bacc.Bacc("TRN2", target_bir_lowering=False, debug=False)
    embT = nc.dram_tensor("embT", [E, B], BF16, kind="ExternalInput").ap()
    wsh = nc.dram_tensor("w_shard", [E, CSP], BF16, kind="ExternalInput").ap()
    wlab = nc.dram_tensor("wlab", [E, B], BF16, kind="ExternalInput").ap()
    out = nc.dram_tensor("out", [CSP, B], BF16, kind="ExternalOutput").ap()
    mv = nc.dram_tensor("mvals", [128, NK], F32, kind="ExternalOutput").ap()
    rnscr = nc.dram_tensor("rnscr", [1, B], F32).ap()
    pfscr = nc.dram_tensor("pfscr", [1, 2 * B], F32).ap()
    with tile.TileContext(nc) as tc, ExitStack() as ctx:
        pools = _make_pools(ctx, tc)
        if reps == 1:
            _build_graph(pools, tc, nc, embT, wsh, wlab, out, mv, rnscr, pfscr)
        else:
            hints = (
                mybir.EngineType.PE,
                mybir.EngineType.DVE,
                mybir.EngineType.Activation,
                mybir.EngineType.SP,
            ) + (() if "nopoolhint" in ABLATE else (mybir.EngineType.Pool,))
            with tc.For_i(0, reps, 1, hint_engines=hints):
                _build_graph(pools, tc, nc, embT, wsh, wlab, out, mv, rnscr, pfscr)
    nc.compile()
    _NC_CACHE[reps] = nc
    return nc


def _prep_inputs(embeddings, labels, w):
    embf = np.asarray(embeddings, dtype=np.float32).astype(ml_dtypes.bfloat16)
    embT = np.ascontiguousarray(embf.T)
    lab = np.asarray(labels).astype(np.int64)
    wf = np.asarray(w, dtype=np.float32)
    wb = wf.astype(ml_dtypes.bfloat16)
    wlab = np.ascontiguousarray(wb[:, lab])
    in_maps = []
    for i in range(NCORES):
        shard = np.zeros((E, CSP), ml_dtypes.bfloat16)
        shard[:, :CSH] = wb[:, i * CSH : (i + 1) * CSH]
        in_maps.append({"embT": embT, "w_shard": shard, "wlab": wlab})
    return lab, in_maps


def _assemble(results, lab):
    out = np.empty((B, C), np.float32)
    for i in range(NCORES):
        out[:, i * CSH : (i + 1) * CSH] = (
            results[i]["out"][:CSH, :].T.astype(np.float32)
        )
    mvals = results[0]["mvals"].T.reshape(B)
    out[np.arange(B), lab] = mvals
    return out


def kernel(embeddings, labels, w):
    nc = _build()
    lab, in_maps = _prep_inputs(embeddings, labels, w)
    r = run_bass_kernel_spmd(nc, in_maps, core_ids=list(range(NCORES)))
    return _assemble(r.results, lab)


def kernel_profiled(embeddings, labels, w, **trace_kwargs):
    """Like kernel() but traces; returns (output, BassKernelResults)."""
    nc = _build()
    lab, in_maps = _prep_inputs(embeddings, labels, w)
    r = run_bass_kernel_spmd(
        nc, in_maps, core_ids=list(range(NCORES)), trace=True, **trace_kwargs
    )
    return _assemble(r.results, lab), r


# revision 10
# speedup vs baseline: 1.0182x; 1.0182x over previous
"""ArcFace logits kernel for 8 TRN2 NeuronCores (class-parallel / Partial-FC style).

Full computation:
    en = l2norm_rows(embeddings)           # [B, E]
    wn = l2norm_cols(w)                    # [E, C]
    cos = clip(en @ wn, -1+1e-6, 1-1e-6)   # [B, C]
    logits = 64 * where(onehot(labels), margin(cos), cos)

Distribution: class dim C=100000 sharded 12500-per-core (padded to 12544 =
98*128). Embeddings replicated. Each core computes its logits shard
transposed ([C_shard, B]: per-column norm scale is a per-partition scalar).

v4 structure (v1 baseline measured 138 us):
- host ships the raw TRANSPOSED embeddings embT [E,B] bf16 (pure
  formatting); no on-device transposes. Row norms are computed FROM embT:
  squared (fp16) then column-summed by ones-stationary matmuls into a
  [1,B] PSUM row; 1/sqrt of that row is partition-broadcast by a tiny
  K=1 matmul into rnB [128,B], and the moving operand eTn = embT * rnB
  is built by 4 DVE multiplies. Group-0 matmuls start on the RAW embT
  (~3us in) with the row norm folded into their drains; all later groups
  stream eTn with plain per-partition-scaled drains.
- one merged DMA per logical transfer: the HWDGE descriptor engine costs
  ~0.6us per DMA serialized across queues, so DMA COUNT is a first-class
  resource (13 HWDGE DMAs vs 86 in v1). Output stores go through the
  otherwise-idle GPSIMD's SWDGE queue.
- w streams in ramped groups [2,4,8,14...] of C-tiles, 3-deep ring, one
  DMA per group.
- column norms: fp16 squared weights (2x DVE throughput vs v1's fp8, and
  better precision); the 4-tiny-MM-per-C-tile partition reductions are
  self-hosted in each group's first tiles, with drains lagging behind a
  6-deep PSUM ring until the group's scale vector is ready.
- margin path: elementwise prod_k = wlab_k * embT_k (fp16) column-summed
  by ones-stationary matmuls into [1,B] PSUM rows, rearranged by a tiny
  DMA into [128,4] for the margin math; cos scaled by s_wl and rn.

dtype: matmuls bf16 with f32 PSUM accumulation; logits bf16. Dense clip
skipped (|cos| << 1-1e-6 for this distribution; the margin path applies
clip exactly). End-to-end rel err vs the f32 reference ~3e-3 (tol 2e-2).
"""

import math
import os
from contextlib import ExitStack

ABLATE = set(os.environ.get("ABLATE4", "").split(","))

import ml_dtypes
import numpy as np

import concourse.bass as bass
import concourse.tile as tile
from concourse import bacc, mybir
from concourse.bass import ts
from concourse.bass_utils import run_bass_kernel_spmd

F32 = mybir.dt.float32
BF16 = mybir.dt.bfloat16
FP16 = mybir.dt.float16
AF = mybir.ActivationFunctionType
ALU = mybir.AluOpType

B = 512          # batch
E = 512          # embedding dim
C = 100000       # classes
NCORES = 8
CSH = C // NCORES          # 12500 real shard width
CSP = 12544                # padded shard width = 98 * 128
NT = CSP // 128            # 98 C-tiles of 128
NK = E // 128              # 4 contraction blocks

GROUPS = [2, 4, 6, 8, 10, 14, 14, 14, 14, 12]   # C-tiles per w-load group
assert sum(GROUPS) == NT
NG = len(GROUPS)
GSTART = [0]
for t in GROUPS:
    GSTART.append(GSTART[-1] + t)
RAW_GROUPS = {0}          # groups whose matmuls stream raw embT (rn in drain)

MARGIN_G = 6     # emit margin block inside this group's tile loop
MARGIN_J = 7

MARGIN = 0.5
SCALE = 64.0
COS_M = math.cos(MARGIN)
SIN_M = math.sin(MARGIN)
TH = math.cos(math.pi - MARGIN)
MM = math.sin(MARGIN) * MARGIN
CLIP_EPS = 1e-6
NORM_EPS = 1e-12


# tiny-MM hosting: which group's tiny units run inside group h's tile loop,
# and over which tail window of h's big-MM slots they spread.
#   ramp groups 0,1 self-host (bunched in their first tiles);
#   from h=1 onward, group h also hosts group h+2's... see _tiny_schedule.
def _tiny_schedule():
    """For each host group h, a dict slot->(list of tiny units).

    A slot is (local_j, k) of a big MM in h; a tiny unit is (g, jj, k).
    Group 0 and 1 self-host bunched at their start; group g>=2 is hosted
    in group g-1, spread over a tail window sized so its w DMA has landed.
    """
    sched = {h: {} for h in range(NG)}

    def place(h, units, window):
        # spread units over the last `window` tiles' big-MM slots of group h
        slots = [(j, k) for j in range(GROUPS[h] - window, GROUPS[h])
                 for k in range(NK)]
        n = len(units)
        m = len(slots)
        for i, u in enumerate(units):
            s = slots[i * m // n]
            sched[h].setdefault(s, []).append(u)

    for g in range(NG):
        units = [(g, jj, k) for jj in range(GROUPS[g]) for k in range(NK)]
        if g == 0:
            place(0, units, GROUPS[0])
        elif g == 1:
            place(1, units, GROUPS[1])
        else:
            h = g - 1
            window = {2: 2, 3: 4}.get(g, GROUPS[h])
            place(h, units, window)
    return sched


def _out_splits(tg, last):
    """Split a group's tiles into out-store staging chunks (<=8 tiles each)."""
    splits = []
    rem = tg
    while rem > 8:
        splits.append(7)
        rem -= 7
    splits.append(rem)
    return splits


def _make_pools(ctx, tc):
    p = {}
    p["sm"] = ctx.enter_context(tc.tile_pool(name="sm", bufs=1))
    p["pw"] = ctx.enter_context(tc.tile_pool(name="pw", bufs=3))
    p["pw2"] = ctx.enter_context(tc.tile_pool(name="pw2", bufs=2))
    p["psd"] = ctx.enter_context(tc.tile_pool(name="psd", bufs=2))
    p["pout"] = ctx.enter_context(tc.tile_pool(name="pout", bufs=4))
    p["psm"] = ctx.enter_context(tc.tile_pool(name="psm", bufs=5, space="PSUM"))
    p["psr"] = ctx.enter_context(tc.tile_pool(name="psr", bufs=1, space="PSUM"))
    p["pscn"] = ctx.enter_context(tc.tile_pool(name="pscn", bufs=2, space="PSUM"))
    return p


def _build_graph(p, tc, nc, embT, wsh, wlab, out, mv, rnscr, pfscr):
    p_sm = p["sm"]

    # --- constants ---
    ones16 = p_sm.tile([128, 1], FP16)
    nc.vector.memset(ones16[:], 1.0)
    onesB = p_sm.tile([1, 128], BF16)
    nc.vector.memset(onesB[:], 1.0)
    eps1 = p_sm.tile([1, 1], F32)
    nc.vector.memset(eps1[:], NORM_EPS)
    eps128 = p_sm.tile([128, 1], F32)
    nc.vector.memset(eps128[:], NORM_EPS)
    # first ACT instruction is a Sqrt so the table pass loads the combined
    # sqrt table (copy+square+sqrt) once instead of reloading mid-kernel
    warm = p_sm.tile([1, 1], F32)
    nc.scalar.activation(warm[:], eps1[:], AF.Sqrt)

    # --- merged DMA prologue ---
    # eTr: raw transposed embeddings [128, NK*B]; slice k = embT[k*128:(k+1)*128, :]
    eTr_all = p_sm.tile([128, NK * B], BF16)
    nc.scalar.dma_start(
        eTr_all[:].rearrange("q (k b) -> q k b", k=NK),
        embT[:, :].rearrange("(k q) b -> q k b", q=128),
    )
    eTr = [eTr_all[:, ts(k, B)] for k in range(NK)]

    wch = {}      # g -> w_all tile [128, NK*width]

    def issue_w_dma(g):
        c0, c1 = GSTART[g] * 128, GSTART[g + 1] * 128
        w_all = p["pw"].tile([128, NK * (c1 - c0)], BF16, name="w_all")
        nc.sync.dma_start(
            w_all[:].rearrange("q (k c) -> q k c", k=NK),
            wsh[:, c0:c1].rearrange("(k q) c -> q k c", q=128),
        )
        wch[g] = w_all

    for g in (0, 1, 2):
        issue_w_dma(g)

    def wsl(g, k, j):
        width = GROUPS[g] * 128
        return wch[g][:, k * width + j * 128 : k * width + (j + 1) * 128]

    # --- row norms from embT: rnB[p, b] = 1/||emb row b||, and eTn = embT*rnB ---
    # squares (fp16, split DVE/ACT), ones-stationary column sums -> [1, B]
    esq = p_sm.tile([128, NK * B], FP16)
    nc.vector.scalar_tensor_tensor(
        esq[:, : 2 * B], eTr_all[:, : 2 * B], 1.0, eTr_all[:, : 2 * B],
        op0=ALU.mult, op1=ALU.mult,
    )
    nc.scalar.activation(esq[:, 2 * B :], eTr_all[:, 2 * B :], AF.Square)
    psE = p["psr"].tile([128, B], F32, name="psr")
    for k in range(NK):
        nc.tensor.matmul(psE[0:1, :], ones16[:], esq[:, ts(k, B)],
                         start=(k == 0), stop=(k == NK - 1))
    rowE = p_sm.tile([1, B], F32)
    nc.scalar.activation(rowE[:], psE[0:1, :], AF.Sqrt, bias=eps1[:])
    rowR = p_sm.tile([1, B], F32)
    nc.vector.reciprocal(rowR[:], rowE[:])
    rowRb = p_sm.tile([1, B], BF16)
    nc.vector.tensor_copy(rowRb[:], rowR[:])
    rnB = p_sm.tile([128, B], F32)
    eTn_all = p_sm.tile([128, NK * B], BF16)
    eTn = [eTn_all[:, ts(k, B)] for k in range(NK)]

    def emit_norm_part2():
        psR = p["psr"].tile([128, B], F32, name="psr")
        nc.tensor.matmul(psR[:], onesB[:], rowRb[:], start=True, stop=True)
        nc.vector.tensor_copy(rnB[:], psR[:])
        for k in range(NK):
            nc.vector.tensor_mul(eTn_all[:, ts(k, B)], eTr[k], rnB[:])

    # margin-layout copy of the row norms: rn[q, m] = rowR[0, m*128+q]
    # (via DRAM: synthesizing a partition dim from SBUF row bytes miscompiles
    # on hardware, so store the row and reload it rearranged)
    nc.sync.dma_start(rnscr[:, :], rowR[:])
    rn = p_sm.tile([128, NK], F32)
    nc.sync.dma_start(
        rn[:], rnscr[:, :].rearrange("o (m q) -> (o q) m", q=128)
    )

    # --- column-norm machinery (fp16 squared weights) ---
    s_dense = p_sm.tile([128, NT], F32)
    w2ch = {}

    def emit_squares(g):
        if "notiny" in ABLATE:
            return
        width = GROUPS[g] * 128
        w2_all = p["pw2"].tile([128, NK * width], FP16, name="w2_all")
        nc.vector.scalar_tensor_tensor(
            w2_all[:, : 2 * width], wch[g][:, : 2 * width], 1.0,
            wch[g][:, : 2 * width], op0=ALU.mult, op1=ALU.mult,
        )
        nc.scalar.activation(
            w2_all[:, 2 * width :], wch[g][:, 2 * width :], AF.Square
        )
        w2ch[g] = w2_all

    pscn_t = {}

    def emit_tiny_unit(g, jj, k):
        if "notiny" in ABLATE:
            return
        if jj == 0 and k == 0:
            pscn_t[g] = p["pscn"].tile([128, GROUPS[g]], F32, name="pscn")
        width = GROUPS[g] * 128
        nc.tensor.matmul(
            pscn_t[g][:, jj : jj + 1],
            w2ch[g][:, k * width + jj * 128 : k * width + (jj + 1) * 128],
            ones16[:],
            start=(k == 0), stop=(k == NK - 1),
        )

    def emit_scale(g):
        if "notiny" in ABLATE:
            if g == 0:
                nc.vector.memset(s_dense[:], 1.0)
            return
        ssq = p["psd"].tile([128, GROUPS[g]], F32, name="ssq")
        nc.scalar.activation(
            ssq[:], pscn_t[g][:], AF.Sqrt, scale=1.0 / (SCALE * SCALE),
            bias=eps128[:],
        )
        nc.vector.reciprocal(s_dense[:, GSTART[g] : GSTART[g + 1]], ssq[:])

    wl_all = p_sm.tile([128, NK * B], BF16)

    def load_wlab():
        nc.scalar.dma_start(
            wl_all[:].rearrange("q (k b) -> q k b", k=NK),
            wlab[:, :].rearrange("(k q) b -> q k b", q=128),
        )

    def emit_margin():
        # cos at label columns: psA[b] = sum_e wl[e,b]*embT[e,b] (raw),
        # psB[b] = sum_e wl[e,b]^2; cos = psA * rsqrt(psB) * rn
        prod_all = p_sm.tile([128, NK * B], FP16)
        wl2_all = p_sm.tile([128, NK * B], FP16)
        for k in range(NK):
            nc.vector.tensor_mul(
                prod_all[:, ts(k, B)], wl_all[:, ts(k, B)], eTr[k]
            )
            nc.scalar.activation(
                wl2_all[:, ts(k, B)], wl_all[:, ts(k, B)], AF.Square
            )
        psA = p["psr"].tile([128, B], F32, name="psr")
        for k in range(NK):
            nc.tensor.matmul(psA[0:1, :], ones16[:], prod_all[:, ts(k, B)],
                             start=(k == 0), stop=(k == NK - 1))
        rowAB = p_sm.tile([1, 2 * B], F32)
        nc.scalar.activation(rowAB[:, :B], psA[0:1, :], AF.Copy)
        psB = p["psr"].tile([128, B], F32, name="psr")
        for k in range(NK):
            nc.tensor.matmul(psB[0:1, :], ones16[:], wl2_all[:, ts(k, B)],
                             start=(k == 0), stop=(k == NK - 1))
        nc.vector.tensor_copy(rowAB[:, B:], psB[0:1, :])
        nc.sync.dma_start(pfscr[:, :], rowAB[:])
        pf = p_sm.tile([128, 2 * NK], F32)
        nc.sync.dma_start(
            pf[:], pfscr[:, :].rearrange("o (m q) -> (o q) m", q=128)
        )
        psA_r, psB_r = pf[:, :NK], pf[:, NK:]

        swl_s = p_sm.tile([128, NK], F32)
        nc.scalar.activation(swl_s[:], psB_r, AF.Sqrt, bias=eps128[:])
        s_wl = p_sm.tile([128, NK], F32)
        nc.vector.reciprocal(s_wl[:], swl_s[:])
        cosu = p_sm.tile([128, NK], F32)
        nc.vector.tensor_mul(cosu[:], psA_r, s_wl[:])
        cos_lab = p_sm.tile([128, NK], F32)
        nc.vector.tensor_mul(cos_lab[:], cosu[:], rn[:])

        cc = p_sm.tile([128, NK], F32)
        nc.vector.tensor_scalar_min(cc[:], cos_lab[:], 1.0 - CLIP_EPS)
        nc.vector.tensor_scalar_max(cc[:], cc[:], -1.0 + CLIP_EPS)
        c2 = p_sm.tile([128, NK], F32)
        nc.scalar.activation(c2[:], cc[:], AF.Square)
        sinv = p_sm.tile([128, NK], F32)
        nc.scalar.activation(sinv[:], c2[:], AF.Sqrt, scale=-1.0, bias=1.0)
        t1 = p_sm.tile([128, NK], F32)
        nc.vector.tensor_scalar_mul(t1[:], cc[:], COS_M)
        cm = p_sm.tile([128, NK], F32)
        nc.vector.scalar_tensor_tensor(
            cm[:], sinv[:], -SIN_M, t1[:], op0=ALU.mult, op1=ALU.add
        )
        alt = p_sm.tile([128, NK], F32)
        nc.vector.tensor_scalar_sub(alt[:], cc[:], MM)
        mk = p_sm.tile([128, NK], mybir.dt.int32)
        nc.vector.tensor_scalar(mk[:], cc[:], TH, None, op0=ALU.is_gt)
        res = p_sm.tile([128, NK], F32)
        nc.vector.tensor_copy(res[:], alt[:])
        nc.vector.copy_predicated(res[:], mk[:], cm[:])
        mvt = p_sm.tile([128, NK], F32)
        nc.vector.tensor_scalar_mul(mvt[:], res[:], SCALE)
        nc.sync.dma_start(mv[:, :], mvt[:])

    # --- main tile loop over groups ---
    sched = _tiny_schedule()
    pending_scale = {}   # g -> remaining tiny units before scale can fire

    for g in range(NG):
        pending_scale[g] = GROUPS[g] * NK

    def emit_units(units):
        for (ug, jj, k) in units:
            if ug not in w2ch:
                emit_squares(ug)
            emit_tiny_unit(ug, jj, k)
            pending_scale[ug] -= 1
            if pending_scale[ug] == 0:
                emit_scale(ug)

    store_q = [0]

    def store(g, seg_w, seg_base, ot):
        t0 = GSTART[g] + seg_base
        dst = out[t0 * 128 : (t0 + seg_w) * 128, :].rearrange(
            "(i q) b -> q i b", q=128
        )
        srcv = ot[:, : seg_w * B].rearrange("q (i b) -> q i b", i=seg_w)
        eng = nc.sync if store_q[0] % 2 == 0 else nc.scalar
        if "poolstores" in ABLATE:
            eng = nc.gpsimd
        store_q[0] += 1
        eng.dma_start(dst, srcv)

    for g in range(NG):
        tg = GROUPS[g]
        if g == 1:
            load_wlab()
        if g >= 1 and g + 2 < NG:
            issue_w_dma(g + 2)
        moving = eTr if g in RAW_GROUPS else eTn
        splits = _out_splits(tg, g == NG - 1)
        split_edge = []
        acc = 0
        for s in splits:
            acc += s
            split_edge.append(acc)
        defer = g <= 1          # drains wait on rnB/eTn-era scale readiness
        psms = []
        ots = {}
        deferred = []
        seg = 0
        seg_base = 0
        for j in range(tg):
            t = GSTART[g] + j
            if j == seg_base:
                seg_w = splits[seg]
                ot = p["pout"].tile([128, 8 * B], BF16, name="ot")
                ots[seg] = (ot, seg_w, seg_base)
            psm = p["psm"].tile([128, B], F32, name="psm")
            psms.append(psm)
            for k in range(NK):
                nc.tensor.matmul(
                    psm[:], wsl(g, k, j), moving[k],
                    start=(k == 0), stop=(k == NK - 1),
                )
                emit_units(sched[g].get((j, k), []))
            if g == MARGIN_G and j == MARGIN_J:
                emit_margin()

            def drain(j=j, t=t, ot=ot, seg_base=seg_base, psm=psm, g=g):
                half = ot[:, ts(j - seg_base, B)]
                if g in RAW_GROUPS:
                    # raw moving operand: fold the row norm into the drain
                    nc.vector.scalar_tensor_tensor(
                        half, psm[:], s_dense[:, t : t + 1], rnB[:],
                        op0=ALU.mult, op1=ALU.mult,
                    )
                elif j % 2 == 0:
                    nc.scalar.activation(half, psm[:], AF.Copy,
                                         scale=s_dense[:, t : t + 1])
                else:
                    nc.vector.tensor_scalar_mul(half, psm[:],
                                                s_dense[:, t : t + 1])

            if defer:
                deferred.append(drain)
            else:
                drain()
            if j == split_edge[seg] - 1:
                if defer:
                    deferred.append(
                        lambda g=g, seg_w=seg_w, seg_base=seg_base, ot=ot: store(
                            g, seg_w, seg_base, ot
                        )
                    )
                else:
                    store(g, seg_w, seg_base, ot)
                seg_base = split_edge[seg]
                seg += 1
        if g == 0:
            emit_norm_part2()
        for fn in deferred:
            fn()


_NC_CACHE = {}


def _build(reps=1):
    """Build + compile. reps>1 wraps the whole body in a HW loop (for timing)."""
    if reps in _NC_CACHE:
        return _NC_CACHE[reps]
    nc = bacc.Bacc("TRN2", target_bir_lowering=False, debug=False)
    embT = nc.dram_tensor("embT", [E, B], BF16, kind="ExternalInput").ap()
    wsh = nc.dram_tensor("w_shard", [E, CSP], BF16, kind="ExternalInput").ap()
    wlab = nc.dram_tensor("wlab", [E, B], BF16, kind="ExternalInput").ap()
    out = nc.dram_tensor("out", [CSP, B], BF16, kind="ExternalOutput").ap()
    mv = nc.dram_tensor("mvals", [128, NK], F32, kind="ExternalOutput").ap()
    rnscr = nc.dram_tensor("rnscr", [1, B], F32).ap()
    pfscr = nc.dram_tensor("pfscr", [1, 2 * B], F32).ap()
    with tile.TileContext(nc) as tc, ExitStack() as ctx:
        pools = _make_pools(ctx, tc)
        if reps == 1:
            _build_graph(pools, tc, nc, embT, wsh, wlab, out, mv, rnscr, pfscr)
        else:
            hints = (
                mybir.EngineType.PE,
                mybir.EngineType.DVE,
                mybir.EngineType.Activation,
                mybir.EngineType.SP,
            ) + (() if "nopoolhint" in ABLATE else (mybir.EngineType.Pool,))
            with tc.For_i(0, reps, 1, hint_engines=hints):
                _build_graph(pools, tc, nc, embT, wsh, wlab, out, mv, rnscr, pfscr)
    nc.compile()
    _NC_CACHE[reps] = nc
    return nc


def _prep_inputs(embeddings, labels, w):
    embf = np.asarray(embeddings, dtype=np.float32).astype(ml_dtypes.bfloat16)
    embT = np.ascontiguousarray(embf.T)
    lab = np.asarray(labels).astype(np.int64)
    wf = np.asarray(w, dtype=np.float32)
    wb = wf.astype(ml_dtypes.bfloat16)
    wlab = np.ascontiguousarray(wb[:, lab])
    in_maps = []
    for i in range(NCORES):
        shard = np.zeros((E, CSP), ml_dtypes.bfloat16)
        shard[:, :CSH] = wb[:, i * CSH : (i + 1) * CSH]
        in_maps.append({"embT": embT, "w_shard": shard, "wlab": wlab})
    return lab, in_maps


def _assemble(results, lab):
    out = np.empty((B, C), np.float32)
    for i in range(NCORES):
        out[:, i * CSH : (i + 1) * CSH] = (
            results[i]["out"][:CSH, :].T.astype(np.float32)
        )
    mvals = results[0]["mvals"].T.reshape(B)
    out[np.arange(B), lab] = mvals
    return out


def kernel(embeddings, labels, w):
    nc = _build()
    lab, in_maps = _prep_inputs(embeddings, labels, w)
    r = run_bass_kernel_spmd(nc, in_maps, core_ids=list(range(NCORES)))
    return _assemble(r.results, lab)


def kernel_profiled(embeddings, labels, w, **trace_kwargs):
    """Like kernel() but traces; returns (output, BassKernelResults)."""
    nc = _build()
    lab, in_maps = _prep_inputs(embeddings, labels, w)
    r = run_bass_kernel_spmd(
        nc, in_maps, core_ids=list(range(NCORES)), trace=True, **trace_kwargs
    )
    return _assemble(r.results, lab), r


# revision 11
# speedup vs baseline: 1.0807x; 1.0614x over previous
"""ArcFace logits kernel for 8 TRN2 NeuronCores (class-parallel / Partial-FC style).

Full computation:
    en = l2norm_rows(embeddings)           # [B, E]
    wn = l2norm_cols(w)                    # [E, C]
    cos = clip(en @ wn, -1+1e-6, 1-1e-6)   # [B, C]
    logits = 64 * where(onehot(labels), margin(cos), cos)

Distribution: class dim C=100000 sharded 12500-per-core (padded to 12544 =
98*128). Embeddings replicated. Each core computes its logits shard
transposed ([C_shard, B]: per-column norm scale is a per-partition scalar).

v4 structure (v1 baseline measured 138 us):
- host ships the raw TRANSPOSED embeddings embT [E,B] bf16 (pure
  formatting); no on-device transposes. Row norms are computed FROM embT:
  squared (fp16) then column-summed by ones-stationary matmuls into a
  [1,B] PSUM row; 1/sqrt of that row is partition-broadcast by a tiny
  K=1 matmul into rnB [128,B], and the moving operand eTn = embT * rnB
  is built by 4 DVE multiplies. Group-0 matmuls start on the RAW embT
  (~3us in) with the row norm folded into their drains; all later groups
  stream eTn with plain per-partition-scaled drains.
- one merged DMA per logical transfer: the HWDGE descriptor engine costs
  ~0.6us per DMA serialized across queues, so DMA COUNT is a first-class
  resource (13 HWDGE DMAs vs 86 in v1). Output stores go through the
  otherwise-idle GPSIMD's SWDGE queue.
- w streams in ramped groups [2,4,8,14...] of C-tiles, 3-deep ring, one
  DMA per group.
- column norms: fp16 squared weights (2x DVE throughput vs v1's fp8, and
  better precision); the 4-tiny-MM-per-C-tile partition reductions are
  self-hosted in each group's first tiles, with drains lagging behind a
  6-deep PSUM ring until the group's scale vector is ready.
- margin path: elementwise prod_k = wlab_k * embT_k (fp16) column-summed
  by ones-stationary matmuls into [1,B] PSUM rows, rearranged by a tiny
  DMA into [128,4] for the margin math; cos scaled by s_wl and rn.

dtype: matmuls bf16 with f32 PSUM accumulation; logits bf16. Dense clip
skipped (|cos| << 1-1e-6 for this distribution; the margin path applies
clip exactly). End-to-end rel err vs the f32 reference ~3e-3 (tol 2e-2).
"""

import math
import os
from contextlib import ExitStack

ABLATE = set(os.environ.get("ABLATE4", "").split(","))

import ml_dtypes
import numpy as np

import concourse.bass as bass
import concourse.tile as tile
from concourse import bacc, mybir
from concourse.bass import ts
from concourse.bass_utils import run_bass_kernel_spmd

F32 = mybir.dt.float32
BF16 = mybir.dt.bfloat16
FP16 = mybir.dt.float16
AF = mybir.ActivationFunctionType
ALU = mybir.AluOpType

B = 512          # batch
E = 512          # embedding dim
C = 100000       # classes
NCORES = 8
CSH = C // NCORES          # 12500 real shard width
CSP = 12544                # padded shard width = 98 * 128
NT = CSP // 128            # 98 C-tiles of 128
NK = E // 128              # 4 contraction blocks

GROUPS = [2, 4, 6, 8, 10, 14, 14, 14, 14, 12]   # C-tiles per w-load group
assert sum(GROUPS) == NT
NG = len(GROUPS)
GSTART = [0]
for t in GROUPS:
    GSTART.append(GSTART[-1] + t)
RAW_GROUPS = {0}          # groups whose matmuls stream raw embT (rn in drain)

MARGIN_G = 6     # emit margin block inside this group's tile loop
MARGIN_J = 7

MARGIN = 0.5
SCALE = 64.0
COS_M = math.cos(MARGIN)
SIN_M = math.sin(MARGIN)
TH = math.cos(math.pi - MARGIN)
MM = math.sin(MARGIN) * MARGIN
CLIP_EPS = 1e-6
NORM_EPS = 1e-12


# tiny-MM hosting: which group's tiny units run inside group h's tile loop,
# and over which tail window of h's big-MM slots they spread.
#   ramp groups 0,1 self-host (bunched in their first tiles);
#   from h=1 onward, group h also hosts group h+2's... see _tiny_schedule.
def _tiny_schedule():
    """For each host group h, a dict slot->(list of tiny units).

    A slot is (local_j, k) of a big MM in h; a tiny unit is (g, jj, k).
    Group 0 and 1 self-host bunched at their start; group g>=2 is hosted
    in group g-1, spread over a tail window sized so its w DMA has landed.
    """
    sched = {h: {} for h in range(NG)}

    def place(h, units, window):
        # spread units over the last `window` tiles' big-MM slots of group h
        slots = [(j, k) for j in range(GROUPS[h] - window, GROUPS[h])
                 for k in range(NK)]
        n = len(units)
        m = len(slots)
        for i, u in enumerate(units):
            s = slots[i * m // n]
            sched[h].setdefault(s, []).append(u)

    for g in range(NG):
        units = [(g, jj, k) for jj in range(GROUPS[g]) for k in range(NK)]
        if g == 0:
            place(0, units, GROUPS[0])
        elif g == 1:
            place(1, units, GROUPS[1])
        else:
            h = g - 1
            window = {2: 2, 3: 4}.get(g, GROUPS[h])
            place(h, units, window)
    return sched


def _out_splits(tg, last):
    """Split a group's tiles into out-store staging chunks (<=8 tiles each)."""
    splits = []
    rem = tg
    while rem > 8:
        splits.append(7)
        rem -= 7
    splits.append(rem)
    return splits


def _make_pools(ctx, tc):
    p = {}
    p["sm"] = ctx.enter_context(tc.tile_pool(name="sm", bufs=1))
    p["pw"] = ctx.enter_context(tc.tile_pool(name="pw", bufs=3))
    p["pw2"] = ctx.enter_context(tc.tile_pool(name="pw2", bufs=2))
    p["psd"] = ctx.enter_context(tc.tile_pool(name="psd", bufs=2))
    p["pout"] = ctx.enter_context(tc.tile_pool(name="pout", bufs=4))
    p["psm"] = ctx.enter_context(tc.tile_pool(name="psm", bufs=5, space="PSUM"))
    p["psr"] = ctx.enter_context(tc.tile_pool(name="psr", bufs=1, space="PSUM"))
    p["pscn"] = ctx.enter_context(tc.tile_pool(name="pscn", bufs=2, space="PSUM"))
    return p


def _build_graph(p, tc, nc, embT, wsh, wlab, out, mv, rnscr, pfscr):
    p_sm = p["sm"]

    # --- constants ---
    ones16 = p_sm.tile([128, 1], FP16)
    nc.vector.memset(ones16[:], 1.0)
    onesB = p_sm.tile([1, 128], BF16)
    nc.vector.memset(onesB[:], 1.0)
    eps1 = p_sm.tile([1, 1], F32)
    nc.vector.memset(eps1[:], NORM_EPS)
    eps128 = p_sm.tile([128, 1], F32)
    nc.vector.memset(eps128[:], NORM_EPS)
    # first ACT instruction is a Sqrt so the table pass loads the combined
    # sqrt table (copy+square+sqrt) once instead of reloading mid-kernel
    warm = p_sm.tile([1, 1], F32)
    nc.scalar.activation(warm[:], eps1[:], AF.Sqrt)

    # --- merged DMA prologue ---
    # eTr: raw transposed embeddings [128, NK*B]; slice k = embT[k*128:(k+1)*128, :]
    eTr_all = p_sm.tile([128, NK * B], BF16)
    nc.scalar.dma_start(
        eTr_all[:].rearrange("q (k b) -> q k b", k=NK),
        embT[:, :].rearrange("(k q) b -> q k b", q=128),
    )
    eTr = [eTr_all[:, ts(k, B)] for k in range(NK)]

    wch = {}      # g -> w_all tile [128, NK*width]

    def issue_w_dma(g):
        c0, c1 = GSTART[g] * 128, GSTART[g + 1] * 128
        w_all = p["pw"].tile([128, NK * (c1 - c0)], BF16, name="w_all")
        nc.sync.dma_start(
            w_all[:].rearrange("q (k c) -> q k c", k=NK),
            wsh[:, c0:c1].rearrange("(k q) c -> q k c", q=128),
        )
        wch[g] = w_all

    for g in (0, 1, 2):
        issue_w_dma(g)

    def wsl(g, k, j):
        width = GROUPS[g] * 128
        return wch[g][:, k * width + j * 128 : k * width + (j + 1) * 128]

    # --- row norms from embT: rnB[p, b] = 1/||emb row b||, and eTn = embT*rnB ---
    # squares (fp16, split DVE/ACT), ones-stationary column sums -> [1, B]
    esq = p_sm.tile([128, NK * B], FP16)
    nc.vector.scalar_tensor_tensor(
        esq[:, : 2 * B], eTr_all[:, : 2 * B], 1.0, eTr_all[:, : 2 * B],
        op0=ALU.mult, op1=ALU.mult,
    )
    nc.scalar.activation(esq[:, 2 * B :], eTr_all[:, 2 * B :], AF.Square)
    psE = p["psr"].tile([128, B], F32, name="psr")
    for k in range(NK):
        nc.tensor.matmul(psE[0:1, :], ones16[:], esq[:, ts(k, B)],
                         start=(k == 0), stop=(k == NK - 1))
    rowE = p_sm.tile([1, B], F32)
    nc.scalar.activation(rowE[:], psE[0:1, :], AF.Sqrt, bias=eps1[:])
    rowR = p_sm.tile([1, B], F32)
    nc.vector.reciprocal(rowR[:], rowE[:])
    rowRb = p_sm.tile([1, B], BF16)
    nc.vector.tensor_copy(rowRb[:], rowR[:])
    rnB = p_sm.tile([128, B], F32)
    eTn_all = p_sm.tile([128, NK * B], BF16)
    eTn = [eTn_all[:, ts(k, B)] for k in range(NK)]

    def emit_norm_part2():
        psR = p["psr"].tile([128, B], F32, name="psr")
        nc.tensor.matmul(psR[:], onesB[:], rowRb[:], start=True, stop=True)
        nc.vector.tensor_copy(rnB[:], psR[:])
        for k in range(NK):
            nc.vector.tensor_mul(eTn_all[:, ts(k, B)], eTr[k], rnB[:])

    # margin-layout copy of the row norms: rn[q, m] = rowR[0, m*128+q]
    # (via DRAM: synthesizing a partition dim from SBUF row bytes miscompiles
    # on hardware, so store the row and reload it rearranged)
    nc.sync.dma_start(rnscr[:, :], rowR[:])
    rn = p_sm.tile([128, NK], F32)
    nc.sync.dma_start(
        rn[:], rnscr[:, :].rearrange("o (m q) -> (o q) m", q=128)
    )

    # --- column-norm machinery (fp16 squared weights) ---
    s_dense = p_sm.tile([128, NT], F32)
    w2ch = {}

    def emit_squares(g):
        if "notiny" in ABLATE:
            return
        width = GROUPS[g] * 128
        w2_all = p["pw2"].tile([128, NK * width], FP16, name="w2_all")
        nc.vector.scalar_tensor_tensor(
            w2_all[:, : 2 * width], wch[g][:, : 2 * width], 1.0,
            wch[g][:, : 2 * width], op0=ALU.mult, op1=ALU.mult,
        )
        nc.scalar.activation(
            w2_all[:, 2 * width :], wch[g][:, 2 * width :], AF.Square
        )
        w2ch[g] = w2_all

    pscn_t = {}

    def emit_tiny_unit(g, jj, k):
        if "notiny" in ABLATE:
            return
        if jj == 0 and k == 0:
            pscn_t[g] = p["pscn"].tile([128, GROUPS[g]], F32, name="pscn")
        width = GROUPS[g] * 128
        nc.tensor.matmul(
            pscn_t[g][:, jj : jj + 1],
            w2ch[g][:, k * width + jj * 128 : k * width + (jj + 1) * 128],
            ones16[:],
            start=(k == 0), stop=(k == NK - 1),
        )

    def emit_scale(g):
        if "notiny" in ABLATE:
            if g == 0:
                nc.vector.memset(s_dense[:], 1.0)
            return
        ssq = p["psd"].tile([128, GROUPS[g]], F32, name="ssq")
        nc.scalar.activation(
            ssq[:], pscn_t[g][:], AF.Sqrt, scale=1.0 / (SCALE * SCALE),
            bias=eps128[:],
        )
        nc.vector.reciprocal(s_dense[:, GSTART[g] : GSTART[g + 1]], ssq[:])

    wl_all = p_sm.tile([128, NK * B], BF16)

    def load_wlab():
        nc.scalar.dma_start(
            wl_all[:].rearrange("q (k b) -> q k b", k=NK),
            wlab[:, :].rearrange("(k q) b -> q k b", q=128),
        )

    def emit_margin():
        # cos at label columns: psA[b] = sum_e wl[e,b]*embT[e,b] (raw),
        # psB[b] = sum_e wl[e,b]^2; cos = psA * rsqrt(psB) * rn
        prod_all = p_sm.tile([128, NK * B], FP16)
        wl2_all = p_sm.tile([128, NK * B], FP16)
        for k in range(NK):
            nc.vector.tensor_mul(
                prod_all[:, ts(k, B)], wl_all[:, ts(k, B)], eTr[k]
            )
            nc.scalar.activation(
                wl2_all[:, ts(k, B)], wl_all[:, ts(k, B)], AF.Square
            )
        psA = p["psr"].tile([128, B], F32, name="psr")
        for k in range(NK):
            nc.tensor.matmul(psA[0:1, :], ones16[:], prod_all[:, ts(k, B)],
                             start=(k == 0), stop=(k == NK - 1))
        rowAB = p_sm.tile([1, 2 * B], F32)
        nc.scalar.activation(rowAB[:, :B], psA[0:1, :], AF.Copy)
        psB = p["psr"].tile([128, B], F32, name="psr")
        for k in range(NK):
            nc.tensor.matmul(psB[0:1, :], ones16[:], wl2_all[:, ts(k, B)],
                             start=(k == 0), stop=(k == NK - 1))
        nc.vector.tensor_copy(rowAB[:, B:], psB[0:1, :])
        nc.sync.dma_start(pfscr[:, :], rowAB[:])
        pf = p_sm.tile([128, 2 * NK], F32)
        nc.sync.dma_start(
            pf[:], pfscr[:, :].rearrange("o (m q) -> (o q) m", q=128)
        )
        psA_r, psB_r = pf[:, :NK], pf[:, NK:]

        swl_s = p_sm.tile([128, NK], F32)
        nc.scalar.activation(swl_s[:], psB_r, AF.Sqrt, bias=eps128[:])
        s_wl = p_sm.tile([128, NK], F32)
        nc.vector.reciprocal(s_wl[:], swl_s[:])
        cosu = p_sm.tile([128, NK], F32)
        nc.vector.tensor_mul(cosu[:], psA_r, s_wl[:])
        cos_lab = p_sm.tile([128, NK], F32)
        nc.vector.tensor_mul(cos_lab[:], cosu[:], rn[:])

        cc = p_sm.tile([128, NK], F32)
        nc.vector.tensor_scalar_min(cc[:], cos_lab[:], 1.0 - CLIP_EPS)
        nc.vector.tensor_scalar_max(cc[:], cc[:], -1.0 + CLIP_EPS)
        c2 = p_sm.tile([128, NK], F32)
        nc.scalar.activation(c2[:], cc[:], AF.Square)
        sinv = p_sm.tile([128, NK], F32)
        nc.scalar.activation(sinv[:], c2[:], AF.Sqrt, scale=-1.0, bias=1.0)
        t1 = p_sm.tile([128, NK], F32)
        nc.vector.tensor_scalar_mul(t1[:], cc[:], COS_M)
        cm = p_sm.tile([128, NK], F32)
        nc.vector.scalar_tensor_tensor(
            cm[:], sinv[:], -SIN_M, t1[:], op0=ALU.mult, op1=ALU.add
        )
        alt = p_sm.tile([128, NK], F32)
        nc.vector.tensor_scalar_sub(alt[:], cc[:], MM)
        mk = p_sm.tile([128, NK], mybir.dt.int32)
        nc.vector.tensor_scalar(mk[:], cc[:], TH, None, op0=ALU.is_gt)
        res = p_sm.tile([128, NK], F32)
        nc.vector.tensor_copy(res[:], alt[:])
        nc.vector.copy_predicated(res[:], mk[:], cm[:])
        mvt = p_sm.tile([128, NK], F32)
        nc.vector.tensor_scalar_mul(mvt[:], res[:], SCALE)
        nc.sync.dma_start(mv[:, :], mvt[:])

    # --- main tile loop over groups ---
    sched = _tiny_schedule()
    pending_scale = {}   # g -> remaining tiny units before scale can fire

    for g in range(NG):
        pending_scale[g] = GROUPS[g] * NK

    def emit_units(units):
        for (ug, jj, k) in units:
            if ug not in w2ch:
                emit_squares(ug)
            emit_tiny_unit(ug, jj, k)
            pending_scale[ug] -= 1
            if pending_scale[ug] == 0:
                emit_scale(ug)

    store_q = [0]

    def store(g, seg_w, seg_base, ot):
        t0 = GSTART[g] + seg_base
        dst = out[t0 * 128 : (t0 + seg_w) * 128, :].rearrange(
            "(i q) b -> q i b", q=128
        )
        srcv = ot[:, : seg_w * B].rearrange("q (i b) -> q i b", i=seg_w)
        eng = nc.gpsimd if "poolstores" in ABLATE else nc.scalar
        store_q[0] += 1
        eng.dma_start(dst, srcv)

    for g in range(NG):
        tg = GROUPS[g]
        if g == 1:
            load_wlab()
        if g >= 1 and g + 2 < NG:
            issue_w_dma(g + 2)
        moving = eTr if g in RAW_GROUPS else eTn
        splits = _out_splits(tg, g == NG - 1)
        split_edge = []
        acc = 0
        for s in splits:
            acc += s
            split_edge.append(acc)
        defer = g <= 1          # drains wait on rnB/eTn-era scale readiness
        psms = []
        ots = {}
        deferred = []
        seg = 0
        seg_base = 0
        for j in range(tg):
            t = GSTART[g] + j
            if j == seg_base:
                seg_w = splits[seg]
                ot = p["pout"].tile([128, 8 * B], BF16, name="ot")
                ots[seg] = (ot, seg_w, seg_base)
            psm = p["psm"].tile([128, B], F32, name="psm")
            psms.append(psm)
            for k in range(NK):
                nc.tensor.matmul(
                    psm[:], wsl(g, k, j), moving[k],
                    start=(k == 0), stop=(k == NK - 1),
                )
                emit_units(sched[g].get((j, k), []))
            if g == MARGIN_G and j == MARGIN_J:
                emit_margin()

            def drain(j=j, t=t, ot=ot, seg_base=seg_base, psm=psm, g=g):
                half = ot[:, ts(j - seg_base, B)]
                if g in RAW_GROUPS:
                    # raw moving operand: fold the row norm into the drain
                    nc.vector.scalar_tensor_tensor(
                        half, psm[:], s_dense[:, t : t + 1], rnB[:],
                        op0=ALU.mult, op1=ALU.mult,
                    )
                elif j % 2 == 0:
                    nc.scalar.activation(half, psm[:], AF.Copy,
                                         scale=s_dense[:, t : t + 1])
                else:
                    nc.vector.tensor_scalar_mul(half, psm[:],
                                                s_dense[:, t : t + 1])

            if defer:
                deferred.append(drain)
            else:
                drain()
            if j == split_edge[seg] - 1:
                if defer:
                    deferred.append(
                        lambda g=g, seg_w=seg_w, seg_base=seg_base, ot=ot: store(
                            g, seg_w, seg_base, ot
                        )
                    )
                else:
                    store(g, seg_w, seg_base, ot)
                seg_base = split_edge[seg]
                seg += 1
        if g == 0:
            emit_norm_part2()
        for fn in deferred:
            fn()


_NC_CACHE = {}


def _build(reps=1):
    """Build + compile. reps>1 wraps the whole body in a HW loop (for timing)."""
    if reps in _NC_CACHE:
        return _NC_CACHE[reps]
    nc = bacc.Bacc("TRN2", target_bir_lowering=False, debug=False)
    embT = nc.dram_tensor("embT", [E, B], BF16, kind="ExternalInput").ap()
    wsh = nc.dram_tensor("w_shard", [E, CSP], BF16, kind="ExternalInput").ap()
    wlab = nc.dram_tensor("wlab", [E, B], BF16, kind="ExternalInput").ap()
    out = nc.dram_tensor("out", [CSP, B], BF16, kind="ExternalOutput").ap()
    mv = nc.dram_tensor("mvals", [128, NK], F32, kind="ExternalOutput").ap()
    rnscr = nc.dram_tensor("rnscr", [1, B], F32).ap()
    pfscr = nc.dram_tensor("pfscr", [1, 2 * B], F32).ap()
    with tile.TileContext(nc) as tc, ExitStack() as ctx:
        pools = _make_pools(ctx, tc)
        if reps == 1:
            _build_graph(pools, tc, nc, embT, wsh, wlab, out, mv, rnscr, pfscr)
        else:
            hints = (
                mybir.EngineType.PE,
                mybir.EngineType.DVE,
                mybir.EngineType.Activation,
                mybir.EngineType.SP,
            ) + (() if "nopoolhint" in ABLATE else (mybir.EngineType.Pool,))
            with tc.For_i(0, reps, 1, hint_engines=hints):
                _build_graph(pools, tc, nc, embT, wsh, wlab, out, mv, rnscr, pfscr)
    nc.compile()
    _NC_CACHE[reps] = nc
    return nc


def _prep_inputs(embeddings, labels, w):
    embf = np.asarray(embeddings, dtype=np.float32).astype(ml_dtypes.bfloat16)
    embT = np.ascontiguousarray(embf.T)
    lab = np.asarray(labels).astype(np.int64)
    wf = np.asarray(w, dtype=np.float32)
    wb = wf.astype(ml_dtypes.bfloat16)
    wlab = np.ascontiguousarray(wb[:, lab])
    in_maps = []
    for i in range(NCORES):
        shard = np.zeros((E, CSP), ml_dtypes.bfloat16)
        shard[:, :CSH] = wb[:, i * CSH : (i + 1) * CSH]
        in_maps.append({"embT": embT, "w_shard": shard, "wlab": wlab})
    return lab, in_maps


def _assemble(results, lab):
    out = np.empty((B, C), np.float32)
    for i in range(NCORES):
        out[:, i * CSH : (i + 1) * CSH] = (
            results[i]["out"][:CSH, :].T.astype(np.float32)
        )
    mvals = results[0]["mvals"].T.reshape(B)
    out[np.arange(B), lab] = mvals
    return out


def kernel(embeddings, labels, w):
    nc = _build()
    lab, in_maps = _prep_inputs(embeddings, labels, w)
    r = run_bass_kernel_spmd(nc, in_maps, core_ids=list(range(NCORES)))
    return _assemble(r.results, lab)


def kernel_profiled(embeddings, labels, w, **trace_kwargs):
    """Like kernel() but traces; returns (output, BassKernelResults)."""
    nc = _build()
    lab, in_maps = _prep_inputs(embeddings, labels, w)
    r = run_bass_kernel_spmd(
        nc, in_maps, core_ids=list(range(NCORES)), trace=True, **trace_kwargs
    )
    return _assemble(r.results, lab), r


# revision 12
# speedup vs baseline: 1.2749x; 1.1796x over previous
"""ArcFace logits kernel for 8 TRN2 NeuronCores (class-parallel / Partial-FC style).

Full computation:
    en = l2norm_rows(embeddings)           # [B, E]
    wn = l2norm_cols(w)                    # [E, C]
    cos = clip(en @ wn, -1+1e-6, 1-1e-6)   # [B, C]
    logits = 64 * where(onehot(labels), margin(cos), cos)

Distribution: class dim C=100000 sharded 12500-per-core (padded to 12544 =
98*128). Embeddings replicated. Each core computes its logits shard
transposed ([C_shard, B]: per-column norm scale is a per-partition scalar).

v4 structure (v1 baseline measured 138 us):
- host ships the raw TRANSPOSED embeddings embT [E,B] bf16 (pure
  formatting); no on-device transposes. Row norms are computed FROM embT:
  squared (fp16) then column-summed by ones-stationary matmuls into a
  [1,B] PSUM row; 1/sqrt of that row is partition-broadcast by a tiny
  K=1 matmul into rnB [128,B], and the moving operand eTn = embT * rnB
  is built by 4 DVE multiplies. Group-0 matmuls start on the RAW embT
  (~3us in) with the row norm folded into their drains; all later groups
  stream eTn with plain per-partition-scaled drains.
- one merged DMA per logical transfer: the HWDGE descriptor engine costs
  ~0.6us per DMA serialized across queues, so DMA COUNT is a first-class
  resource (13 HWDGE DMAs vs 86 in v1). Output stores go through the
  otherwise-idle GPSIMD's SWDGE queue.
- w streams in ramped groups [2,4,8,14...] of C-tiles, 3-deep ring, one
  DMA per group.
- column norms: fp16 squared weights (2x DVE throughput vs v1's fp8, and
  better precision); the 4-tiny-MM-per-C-tile partition reductions are
  self-hosted in each group's first tiles, with drains lagging behind a
  6-deep PSUM ring until the group's scale vector is ready.
- margin path: elementwise prod_k = wlab_k * embT_k (fp16) column-summed
  by ones-stationary matmuls into [1,B] PSUM rows, rearranged by a tiny
  DMA into [128,4] for the margin math; cos scaled by s_wl and rn.

dtype: matmuls bf16 with f32 PSUM accumulation; logits bf16. Dense clip
skipped (|cos| << 1-1e-6 for this distribution; the margin path applies
clip exactly). End-to-end rel err vs the f32 reference ~3e-3 (tol 2e-2).
"""

import math
import os
from contextlib import ExitStack

ABLATE = set(os.environ.get("ABLATE4", "").split(","))

import ml_dtypes
import numpy as np

import concourse.bass as bass
import concourse.tile as tile
from concourse import bacc, mybir
from concourse.bass import ts
from concourse.bass_utils import run_bass_kernel_spmd

F32 = mybir.dt.float32
BF16 = mybir.dt.bfloat16
FP16 = mybir.dt.float16
AF = mybir.ActivationFunctionType
ALU = mybir.AluOpType

B = 512          # batch
E = 512          # embedding dim
C = 100000       # classes
NCORES = 8
CSH = C // NCORES          # 12500 real shard width
CSP = 12544                # padded shard width = 98 * 128
NT = CSP // 128            # 98 C-tiles of 128
NK = E // 128              # 4 contraction blocks

GROUPS = [2, 4, 6, 8, 10, 14, 14, 14, 14, 12]   # C-tiles per w-load group
assert sum(GROUPS) == NT
NG = len(GROUPS)
GSTART = [0]
for t in GROUPS:
    GSTART.append(GSTART[-1] + t)
RAW_GROUPS = {0}          # groups whose matmuls stream raw embT (rn in drain)

MARGIN_G = 6     # emit margin block inside this group's tile loop
MARGIN_J = 7

MARGIN = 0.5
SCALE = 64.0
COS_M = math.cos(MARGIN)
SIN_M = math.sin(MARGIN)
TH = math.cos(math.pi - MARGIN)
MM = math.sin(MARGIN) * MARGIN
CLIP_EPS = 1e-6
NORM_EPS = 1e-12


# tiny-MM hosting: which group's tiny units run inside group h's tile loop,
# and over which tail window of h's big-MM slots they spread.
#   ramp groups 0,1 self-host (bunched in their first tiles);
#   from h=1 onward, group h also hosts group h+2's... see _tiny_schedule.
def _tiny_schedule():
    """For each host group h, a dict slot->(list of tiny units).

    A slot is (local_j, k) of a big MM in h; a tiny unit is (g, jj, k).
    Group 0 and 1 self-host bunched at their start; group g>=2 is hosted
    in group g-1, spread over a tail window sized so its w DMA has landed.
    """
    sched = {h: {} for h in range(NG)}

    def place(h, units, window):
        # spread units over the last `window` tiles' big-MM slots of group h
        slots = [(j, k) for j in range(GROUPS[h] - window, GROUPS[h])
                 for k in range(NK)]
        n = len(units)
        m = len(slots)
        for i, u in enumerate(units):
            s = slots[i * m // n]
            sched[h].setdefault(s, []).append(u)

    for g in range(NG):
        units = [(g, jj, k) for jj in range(GROUPS[g]) for k in range(NK)]
        if g == 0:
            place(0, units, GROUPS[0])
        elif g == 1:
            place(1, units, GROUPS[1])
        else:
            h = g - 1
            window = {2: 2, 3: 4}.get(g, GROUPS[h])
            place(h, units, window)
    return sched


def _out_splits(tg, last):
    """Split a group's tiles into out-store staging chunks (<=8 tiles each)."""
    splits = []
    rem = tg
    while rem > 8:
        splits.append(7)
        rem -= 7
    splits.append(rem)
    return splits


def _make_pools(ctx, tc):
    p = {}
    p["sm"] = ctx.enter_context(tc.tile_pool(name="sm", bufs=1))
    p["pw"] = ctx.enter_context(tc.tile_pool(name="pw", bufs=3))
    p["pw2"] = ctx.enter_context(tc.tile_pool(name="pw2", bufs=2))
    p["psd"] = ctx.enter_context(tc.tile_pool(name="psd", bufs=2))
    p["pout"] = ctx.enter_context(tc.tile_pool(name="pout", bufs=4))
    p["psm"] = ctx.enter_context(tc.tile_pool(name="psm", bufs=5, space="PSUM"))
    p["psr"] = ctx.enter_context(tc.tile_pool(name="psr", bufs=1, space="PSUM"))
    p["pscn"] = ctx.enter_context(tc.tile_pool(name="pscn", bufs=2, space="PSUM"))
    return p


def _build_graph(p, tc, nc, embT, wsh, wlab, out, mv, rnscr, pfscr):
    p_sm = p["sm"]

    # --- constants ---
    ones16 = p_sm.tile([128, 1], FP16)
    nc.vector.memset(ones16[:], 1.0)
    onesB = p_sm.tile([1, 128], BF16)
    nc.vector.memset(onesB[:], 1.0)
    eps1 = p_sm.tile([1, 1], F32)
    nc.vector.memset(eps1[:], NORM_EPS)
    eps128 = p_sm.tile([128, 1], F32)
    nc.vector.memset(eps128[:], NORM_EPS)
    # first ACT instruction is a Sqrt so the table pass loads the combined
    # sqrt table (copy+square+sqrt) once instead of reloading mid-kernel
    warm = p_sm.tile([1, 1], F32)
    nc.scalar.activation(warm[:], eps1[:], AF.Sqrt)

    # --- merged DMA prologue ---
    # eTr: raw transposed embeddings [128, NK*B]; slice k = embT[k*128:(k+1)*128, :]
    eTr_all = p_sm.tile([128, NK * B], BF16)
    nc.scalar.dma_start(
        eTr_all[:].rearrange("q (k b) -> q k b", k=NK),
        embT[:, :].rearrange("(k q) b -> q k b", q=128),
    )
    eTr = [eTr_all[:, ts(k, B)] for k in range(NK)]

    wch = {}      # g -> w_all tile [128, NK*width]

    def issue_w_dma(g):
        c0, c1 = GSTART[g] * 128, GSTART[g + 1] * 128
        w_all = p["pw"].tile([128, NK * (c1 - c0)], BF16, name="w_all")
        nc.sync.dma_start(
            w_all[:].rearrange("q (k c) -> q k c", k=NK),
            wsh[:, c0:c1].rearrange("(k q) c -> q k c", q=128),
        )
        wch[g] = w_all

    for g in (0, 1, 2):
        issue_w_dma(g)

    def wsl(g, k, j):
        width = GROUPS[g] * 128
        return wch[g][:, k * width + j * 128 : k * width + (j + 1) * 128]

    # --- row norms from embT: rnB[p, b] = 1/||emb row b||, and eTn = embT*rnB ---
    # squares (fp16, split DVE/ACT), ones-stationary column sums -> [1, B]
    NONORM = "nonorm" in ABLATE
    esq = p_sm.tile([128, NK * B], FP16)
    if not NONORM:
        nc.vector.scalar_tensor_tensor(
        esq[:, : 2 * B], eTr_all[:, : 2 * B], 1.0, eTr_all[:, : 2 * B],
        op0=ALU.mult, op1=ALU.mult,
        )
    rowE = p_sm.tile([1, B], F32)
    rowR = p_sm.tile([1, B], F32)
    rowRb = p_sm.tile([1, B], BF16)
    if not NONORM:
        nc.scalar.activation(esq[:, 2 * B :], eTr_all[:, 2 * B :], AF.Square)
        psE = p["psr"].tile([128, B], F32, name="psr")
        for k in range(NK):
            nc.tensor.matmul(psE[0:1, :], ones16[:], esq[:, ts(k, B)],
                             start=(k == 0), stop=(k == NK - 1))
        nc.scalar.activation(rowE[:], psE[0:1, :], AF.Sqrt, bias=eps1[:])
        nc.vector.reciprocal(rowR[:], rowE[:])
        nc.vector.tensor_copy(rowRb[:], rowR[:])
    rnB = p_sm.tile([128, B], F32)
    eTn_all = p_sm.tile([128, NK * B], BF16)
    eTn = [eTn_all[:, ts(k, B)] for k in range(NK)]

    def emit_norm_part2():
        if NONORM:
            return
        psR = p["psr"].tile([128, B], F32, name="psr")
        nc.tensor.matmul(psR[:], onesB[:], rowRb[:], start=True, stop=True)
        nc.vector.tensor_copy(rnB[:], psR[:])
        for k in range(NK):
            nc.vector.tensor_mul(eTn_all[:, ts(k, B)], eTr[k], rnB[:])

    # margin-layout copy of the row norms: rn[q, m] = rowR[0, m*128+q]
    # (via DRAM: synthesizing a partition dim from SBUF row bytes miscompiles
    # on hardware, so store the row and reload it rearranged)
    rn = p_sm.tile([128, NK], F32)
    if not NONORM:
        nc.sync.dma_start(rnscr[:, :], rowR[:])
        nc.sync.dma_start(
            rn[:], rnscr[:, :].rearrange("o (m q) -> (o q) m", q=128)
        )

    # --- column-norm machinery (fp16 squared weights) ---
    s_dense = p_sm.tile([128, NT], F32)
    w2ch = {}

    def emit_squares(g):
        if "notiny" in ABLATE:
            return
        width = GROUPS[g] * 128
        w2_all = p["pw2"].tile([128, NK * width], FP16, name="w2_all")
        nc.vector.scalar_tensor_tensor(
            w2_all[:, : 2 * width], wch[g][:, : 2 * width], 1.0,
            wch[g][:, : 2 * width], op0=ALU.mult, op1=ALU.mult,
        )
        nc.scalar.activation(
            w2_all[:, 2 * width :], wch[g][:, 2 * width :], AF.Square
        )
        w2ch[g] = w2_all

    pscn_t = {}

    def emit_tiny_unit(g, jj, k):
        if "notiny" in ABLATE:
            return
        if jj == 0 and k == 0:
            pscn_t[g] = p["pscn"].tile([128, GROUPS[g]], F32, name="pscn")
        width = GROUPS[g] * 128
        nc.tensor.matmul(
            pscn_t[g][:, jj : jj + 1],
            w2ch[g][:, k * width + jj * 128 : k * width + (jj + 1) * 128],
            ones16[:],
            start=(k == 0), stop=(k == NK - 1),
        )

    def emit_scale(g):
        if "notiny" in ABLATE:
            if g == 0:
                nc.vector.memset(s_dense[:], 1.0)
            return
        ssq = p["psd"].tile([128, GROUPS[g]], F32, name="ssq")
        nc.scalar.activation(
            ssq[:], pscn_t[g][:], AF.Sqrt, scale=1.0 / (SCALE * SCALE),
            bias=eps128[:],
        )
        nc.vector.reciprocal(s_dense[:, GSTART[g] : GSTART[g + 1]], ssq[:])

    wl_all = p_sm.tile([128, NK * B], BF16)

    def load_wlab():
        nc.scalar.dma_start(
            wl_all[:].rearrange("q (k b) -> q k b", k=NK),
            wlab[:, :].rearrange("(k q) b -> q k b", q=128),
        )

    def emit_margin():
        # cos at label columns: psA[b] = sum_e wl[e,b]*embT[e,b] (raw),
        # psB[b] = sum_e wl[e,b]^2; cos = psA * rsqrt(psB) * rn
        prod_all = p_sm.tile([128, NK * B], FP16)
        wl2_all = p_sm.tile([128, NK * B], FP16)
        for k in range(NK):
            nc.vector.tensor_mul(
                prod_all[:, ts(k, B)], wl_all[:, ts(k, B)], eTr[k]
            )
            nc.scalar.activation(
                wl2_all[:, ts(k, B)], wl_all[:, ts(k, B)], AF.Square
            )
        psA = p["psr"].tile([128, B], F32, name="psr")
        for k in range(NK):
            nc.tensor.matmul(psA[0:1, :], ones16[:], prod_all[:, ts(k, B)],
                             start=(k == 0), stop=(k == NK - 1))
        rowAB = p_sm.tile([1, 2 * B], F32)
        nc.scalar.activation(rowAB[:, :B], psA[0:1, :], AF.Copy)
        psB = p["psr"].tile([128, B], F32, name="psr")
        for k in range(NK):
            nc.tensor.matmul(psB[0:1, :], ones16[:], wl2_all[:, ts(k, B)],
                             start=(k == 0), stop=(k == NK - 1))
        nc.vector.tensor_copy(rowAB[:, B:], psB[0:1, :])
        nc.sync.dma_start(pfscr[:, :], rowAB[:])
        pf = p_sm.tile([128, 2 * NK], F32)
        nc.sync.dma_start(
            pf[:], pfscr[:, :].rearrange("o (m q) -> (o q) m", q=128)
        )
        psA_r, psB_r = pf[:, :NK], pf[:, NK:]

        swl_s = p_sm.tile([128, NK], F32)
        nc.scalar.activation(swl_s[:], psB_r, AF.Sqrt, bias=eps128[:])
        s_wl = p_sm.tile([128, NK], F32)
        nc.vector.reciprocal(s_wl[:], swl_s[:])
        cosu = p_sm.tile([128, NK], F32)
        nc.vector.tensor_mul(cosu[:], psA_r, s_wl[:])
        cos_lab = p_sm.tile([128, NK], F32)
        nc.vector.tensor_mul(cos_lab[:], cosu[:], rn[:])

        cc = p_sm.tile([128, NK], F32)
        nc.vector.tensor_scalar_min(cc[:], cos_lab[:], 1.0 - CLIP_EPS)
        nc.vector.tensor_scalar_max(cc[:], cc[:], -1.0 + CLIP_EPS)
        c2 = p_sm.tile([128, NK], F32)
        nc.scalar.activation(c2[:], cc[:], AF.Square)
        sinv = p_sm.tile([128, NK], F32)
        nc.scalar.activation(sinv[:], c2[:], AF.Sqrt, scale=-1.0, bias=1.0)
        t1 = p_sm.tile([128, NK], F32)
        nc.vector.tensor_scalar_mul(t1[:], cc[:], COS_M)
        cm = p_sm.tile([128, NK], F32)
        nc.vector.scalar_tensor_tensor(
            cm[:], sinv[:], -SIN_M, t1[:], op0=ALU.mult, op1=ALU.add
        )
        alt = p_sm.tile([128, NK], F32)
        nc.vector.tensor_scalar_sub(alt[:], cc[:], MM)
        mk = p_sm.tile([128, NK], mybir.dt.int32)
        nc.vector.tensor_scalar(mk[:], cc[:], TH, None, op0=ALU.is_gt)
        res = p_sm.tile([128, NK], F32)
        nc.vector.tensor_copy(res[:], alt[:])
        nc.vector.copy_predicated(res[:], mk[:], cm[:])
        mvt = p_sm.tile([128, NK], F32)
        nc.vector.tensor_scalar_mul(mvt[:], res[:], SCALE)
        nc.sync.dma_start(mv[:, :], mvt[:])

    # --- main tile loop over groups ---
    sched = _tiny_schedule()
    pending_scale = {}   # g -> remaining tiny units before scale can fire

    for g in range(NG):
        pending_scale[g] = GROUPS[g] * NK

    def emit_units(units):
        for (ug, jj, k) in units:
            if ug not in w2ch:
                emit_squares(ug)
            emit_tiny_unit(ug, jj, k)
            pending_scale[ug] -= 1
            if pending_scale[ug] == 0:
                emit_scale(ug)

    store_q = [0]

    def store(g, seg_w, seg_base, ot):
        t0 = GSTART[g] + seg_base
        dst = out[t0 * 128 : (t0 + seg_w) * 128, :].rearrange(
            "(i q) b -> q i b", q=128
        )
        srcv = ot[:, : seg_w * B].rearrange("q (i b) -> q i b", i=seg_w)
        eng = nc.gpsimd if "poolstores" in ABLATE else nc.scalar
        store_q[0] += 1
        eng.dma_start(dst, srcv)

    for g in range(NG):
        tg = GROUPS[g]
        if g == 1:
            load_wlab()
        if g >= 1 and g + 2 < NG:
            issue_w_dma(g + 2)
        moving = eTr if (NONORM or g in RAW_GROUPS) else eTn
        splits = _out_splits(tg, g == NG - 1)
        split_edge = []
        acc = 0
        for s in splits:
            acc += s
            split_edge.append(acc)
        defer = g <= 1          # drains wait on rnB/eTn-era scale readiness
        psms = []
        ots = {}
        deferred = []
        seg = 0
        seg_base = 0
        for j in range(tg):
            t = GSTART[g] + j
            if j == seg_base:
                seg_w = splits[seg]
                ot = p["pout"].tile([128, 8 * B], BF16, name="ot")
                ots[seg] = (ot, seg_w, seg_base)
            psm = p["psm"].tile([128, B], F32, name="psm")
            psms.append(psm)
            for k in range(NK):
                nc.tensor.matmul(
                    psm[:], wsl(g, k, j), moving[k],
                    start=(k == 0), stop=(k == NK - 1),
                )
                emit_units(sched[g].get((j, k), []))
            if g == MARGIN_G and j == MARGIN_J and "nomargin" not in ABLATE:
                emit_margin()

            def drain(j=j, t=t, ot=ot, seg_base=seg_base, psm=psm, g=g):
                half = ot[:, ts(j - seg_base, B)]
                if g in RAW_GROUPS and not NONORM:
                    # raw moving operand: fold the row norm into the drain
                    nc.vector.scalar_tensor_tensor(
                        half, psm[:], s_dense[:, t : t + 1], rnB[:],
                        op0=ALU.mult, op1=ALU.mult,
                    )
                elif j % 2 == 0:
                    nc.scalar.activation(half, psm[:], AF.Copy,
                                         scale=s_dense[:, t : t + 1])
                else:
                    nc.vector.tensor_scalar_mul(half, psm[:],
                                                s_dense[:, t : t + 1])

            if defer:
                deferred.append(drain)
            else:
                drain()
            if j == split_edge[seg] - 1:
                if defer:
                    deferred.append(
                        lambda g=g, seg_w=seg_w, seg_base=seg_base, ot=ot: store(
                            g, seg_w, seg_base, ot
                        )
                    )
                else:
                    store(g, seg_w, seg_base, ot)
                seg_base = split_edge[seg]
                seg += 1
        if g == 0:
            emit_norm_part2()
        for fn in deferred:
            fn()


_NC_CACHE = {}


def _build(reps=1):
    """Build + compile. reps>1 wraps the whole body in a HW loop (for timing)."""
    if reps in _NC_CACHE:
        return _NC_CACHE[reps]
    nc = bacc.Bacc("TRN2", target_bir_lowering=False, debug=False)
    embT = nc.dram_tensor("embT", [E, B], BF16, kind="ExternalInput").ap()
    wsh = nc.dram_tensor("w_shard", [E, CSP], BF16, kind="ExternalInput").ap()
    wlab = nc.dram_tensor("wlab", [E, B], BF16, kind="ExternalInput").ap()
    out = nc.dram_tensor("out", [CSP, B], BF16, kind="ExternalOutput").ap()
    mv = nc.dram_tensor("mvals", [128, NK], F32, kind="ExternalOutput").ap()
    rnscr = nc.dram_tensor("rnscr", [1, B], F32).ap()
    pfscr = nc.dram_tensor("pfscr", [1, 2 * B], F32).ap()
    with tile.TileContext(nc) as tc, ExitStack() as ctx:
        pools = _make_pools(ctx, tc)
        if reps == 1:
            _build_graph(pools, tc, nc, embT, wsh, wlab, out, mv, rnscr, pfscr)
        else:
            hints = (
                mybir.EngineType.PE,
                mybir.EngineType.DVE,
                mybir.EngineType.Activation,
                mybir.EngineType.SP,
            ) + (() if "nopoolhint" in ABLATE else (mybir.EngineType.Pool,))
            with tc.For_i(0, reps, 1, hint_engines=hints):
                _build_graph(pools, tc, nc, embT, wsh, wlab, out, mv, rnscr, pfscr)
    nc.compile()
    _NC_CACHE[reps] = nc
    return nc


def _prep_inputs(embeddings, labels, w):
    embf = np.asarray(embeddings, dtype=np.float32).astype(ml_dtypes.bfloat16)
    embT = np.ascontiguousarray(embf.T)
    lab = np.asarray(labels).astype(np.int64)
    wf = np.asarray(w, dtype=np.float32)
    wb = wf.astype(ml_dtypes.bfloat16)
    wlab = np.ascontiguousarray(wb[:, lab])
    in_maps = []
    for i in range(NCORES):
        shard = np.zeros((E, CSP), ml_dtypes.bfloat16)
        shard[:, :CSH] = wb[:, i * CSH : (i + 1) * CSH]
        in_maps.append({"embT": embT, "w_shard": shard, "wlab": wlab})
    return lab, in_maps


def _assemble(results, lab):
    out = np.empty((B, C), np.float32)
    for i in range(NCORES):
        out[:, i * CSH : (i + 1) * CSH] = (
            results[i]["out"][:CSH, :].T.astype(np.float32)
        )
    mvals = results[0]["mvals"].T.reshape(B)
    out[np.arange(B), lab] = mvals
    return out


def kernel(embeddings, labels, w):
    nc = _build()
    lab, in_maps = _prep_inputs(embeddings, labels, w)
    r = run_bass_kernel_spmd(nc, in_maps, core_ids=list(range(NCORES)))
    return _assemble(r.results, lab)


def kernel_profiled(embeddings, labels, w, **trace_kwargs):
    """Like kernel() but traces; returns (output, BassKernelResults)."""
    nc = _build()
    lab, in_maps = _prep_inputs(embeddings, labels, w)
    r = run_bass_kernel_spmd(
        nc, in_maps, core_ids=list(range(NCORES)), trace=True, **trace_kwargs
    )
    return _assemble(r.results, lab), r
